# revision 1
# baseline (speedup 1.0000x reference)
"""Trainium2 Bass kernel for a transformer decoder layer — 8-way tensor parallel.

Sharding (per sharding_hint): tensor-parallel over the 16 attention heads
(2 heads/core) and the 4096-dim FFN hidden (512/core), with sequence-sharded
residual+LayerNorm (Megatron sequence-parallel style):

  - token shard: core r owns rows [512r, 512(r+1)) of the flattened
    [B*L, D] = [4096, 1024] token axis (batch 0 = cores 0-3, batch 1 = 4-7).
  - channel shard: core r owns head-major channels [128r, 128(r+1))
    (= heads 2r, 2r+1) of every attention projection, and hidden slice
    [512r, 512(r+1)) of the FFN.

Data flow per core:
  transpose own x shard -> AG -> x^T full   (critical-path collective, first;
  AG(compacted enc^T) queues behind it and completes during self-attention,
  where the cross-attention K projection is also hoisted)
  Q/K proj (own 128 channels, all 4096 tokens) -> transposed-score attention
  (softmax denominator via ones-column, reference's K-as-V kept) -> partial
  o-proj [4096, 1024] -> ReduceScatter(add) -> own 512 rows summed ->
  +residual -> LN -> next phase (cross-attn, then FFN).

Cross-attention context is compacted host-side: masked enc positions have
softmax weight exactly 0 in the reference (exp(-1.25e8) == 0), so only
unmasked rows are kept (padded per batch to a multiple of 128; pad slots
re-masked via the exp bias) — exact, and drops ~45% of the cross-attention
score/softmax/value work plus ~40% of the enc upload for a random 0/1 mask.

All collectives are split into two d-halves and pipelined: each half's
ReduceScatter overlaps the other half's o-proj/FFN staging, and each half's
AllGather overlaps the other half's transposes; projection consumers start
on half 0 while half 1 is still in flight.

Numerics: weights and x^T/enc^T gathers in bf16; Q/K/scores/softmax in
fp32(r); context/FFN operands bf16; partial-sum ReduceScatter in bf16;
LayerNorm fp32.  Everything accumulates in fp32 PSUM.

Upload per core ~5.6MB (vs ~79MB replicated data-parallel) — weights are
uploaded exactly once across the 8 cores.
"""

import os
import sys

sys.path.insert(0, "/opt/trn_rl_repo")

import numpy as np

import concourse.bass as bass
import concourse.bacc as bacc
import concourse.mybir as mybir
import concourse.tile as tile
from concourse.bass_utils import run_bass_kernel_spmd
from concourse.masks import make_identity

dt = mybir.dt
AF = mybir.ActivationFunctionType
ALU = mybir.AluOpType

P = 128
D = 1024          # d_model
H = 16            # heads
HD = 64           # head dim
MLP = 4096
B, L, M = 2, 2048, 2048
T = B * L         # 4096 flat tokens
NCORES = 8
TS = T // NCORES  # 512 tokens per core
CS = D // NCORES  # 128 channels per core (2 heads)
MS = MLP // NCORES  # 512 hidden per core
NK = 512          # matmul free-dim chunk
TC = T // NK      # 8 token chunks
DTL = D // P      # 8 d-tiles
DH = D // 2       # d-half (collective split granularity)
DTH = DTL // 2    # 4 d-tiles per half
MT = M // P       # 16 m-tiles per batch
LT = TS // P      # 4 row-tiles per core shard
HTR = MS // P     # 4 hidden tiles per core
EPS = 1e-5
SQRT1_2 = 0.7071067811865476
RG = [list(range(NCORES))]

_PROGRAM_CACHE = {}


def _build_program(trivial_affine, trivial_ffb, n0p, n1p):
    # n0p/n1p: per-batch compacted+padded cross-attention context lengths
    # (multiples of 128).  Masked enc positions have softmax weight exactly 0
    # in the reference, so dropping them host-side is exact; padded slots are
    # forced to 0 via the exp bias.
    TKC = n0p + n1p          # total compacted context tokens
    SC = TKC // NCORES       # compacted tokens per core shard
    TT = TKC // P            # compacted m-tiles
    nc = bacc.Bacc(None)
    f32 = dt.float32
    rdt = dt.float32r   # full-rate fp32 for score/value path
    b16 = dt.bfloat16

    def din(name, shape, d):
        return nc.declare_dram_parameter(name, list(shape), d, isOutput=False)

    xs_d = din("xs", [TS, D], b16)            # own x rows (residual source)
    encsT_d = din("encsT", [D, SC], b16)      # own compacted-enc shard, transposed
    maskb_d = din("maskb", [P, TT], f32)      # -1.25e8 on padded slots else 0
    q1W_d = din("q1W", [P, DTL * CS], b16)    # partition-major [128, 8, 128]
    w1W_d = din("w1W", [P, DTL * CS], b16)
    o1W_d = din("o1W", [CS, D], b16)          # rows slice, natural
    q2W_d = din("q2W", [P, DTL * CS], b16)
    w2W_d = din("w2W", [P, DTL * CS], b16)
    o2W_d = din("o2W", [CS, D], b16)
    ffW1_d = din("ffW1", [P, DTL * MS], b16)  # [128, 8, 512]
    ffW2_d = din("ffW2", [P, HTR * D], b16)   # [128, 4, 1024]
    gb_d = {}
    if not trivial_affine:
        for nm in ("g1", "b1", "g2", "b2", "g3", "b3"):
            gb_d[nm] = din(nm + "b", [P, D], f32)
    if not trivial_ffb:
        ffb2b_d = din("ffb2b", [P, D], f32)
        ffb1h_d = din("ffb1h", [P, HTR], f32)
        ffb1e_d = din("ffb1e", [P, HTR], f32)
    out_d = nc.declare_dram_parameter("out", [TS, D], b16, isOutput=True)

    # collective bounce buffers (internal DRAM), split into two d-halves so
    # each half's collective overlaps the other half's compute
    def ag_pair(nm):
        return (
            [nc.dram_tensor(f"{nm}_in{h}", [DH, TS], b16, kind="Internal")
             for h in range(2)],
            [nc.dram_tensor(f"{nm}_out{h}", [NCORES * DH, TS], b16,
                            kind="Internal", addr_space="Shared")
             for h in range(2)],
        )

    agenc_in = [nc.dram_tensor(f"agenc_in{h}", [DH, SC], b16, kind="Internal")
                for h in range(2)]
    agenc_out = [nc.dram_tensor(f"agenc_out{h}", [NCORES * DH, SC], b16,
                                kind="Internal", addr_space="Shared")
                 for h in range(2)]
    agx = [ag_pair(f"agx{i}") for i in range(3)]
    rs_in = [[nc.dram_tensor(f"rs{i}_in{h}", [T, DH], b16, kind="Internal")
              for h in range(2)] for i in range(3)]
    rs_out = [[nc.dram_tensor(f"rs{i}_out{h}", [TS, DH], b16, kind="Internal")
               for h in range(2)] for i in range(3)]

    nocoll = os.environ.get("KTP_NOCOLL", "0") == "1"  # sim-only diagnostic

    def allgather(src, dst):
        if nocoll:
            nc.sync.dma_start(dst[0:src.shape[0], :], src[:])
            return
        nc.gpsimd.collective_compute(
            "AllGather", ALU.bypass, replica_groups=RG, ins=[src[:]], outs=[dst[:]])

    def reducescatter(src, dst):
        if nocoll:
            nc.sync.dma_start(dst[:], src[0:dst.shape[0], :])
            return
        nc.gpsimd.collective_compute(
            "ReduceScatter", ALU.add, replica_groups=RG, ins=[src[:]], outs=[dst[:]])

    lp = nc.allow_low_precision(reason="bf16 weights/activations staging")
    lp.__enter__()
    with tile.TileContext(nc) as tc:
        cpool = tc.alloc_tile_pool(name="const", bufs=1)
        small = tc.alloc_tile_pool(name="small", bufs=3)
        sbP = tc.alloc_tile_pool(name="sbP", bufs=1)

        ident_f = cpool.tile([P, P], f32)
        make_identity(nc, ident_f[:])
        ident_r = cpool.tile([P, P], rdt)
        nc.vector.tensor_copy(ident_r[:], ident_f[:])
        ones_f = cpool.tile([1, HD], f32)
        nc.vector.memset(ones_f[:], 1.0)
        # [P, 4, 2, 64] pad block for K-natural tiles: col0 = 1 (softmax
        # denominator ones column), rest 0
        ozcol = cpool.tile([P, 4, 2, HD], f32)
        nc.vector.memset(ozcol[:], 0.0)
        nc.vector.memset(ozcol[:, :, :, 0:1], 1.0)
        maskb_t = cpool.tile([P, TT], f32)
        nc.sync.dma_start(maskb_t[:], maskb_d[:])
        if not trivial_ffb:
            ffb1h_t = cpool.tile([P, HTR], f32)
            nc.sync.dma_start(ffb1h_t[:], ffb1h_d[:])
            ffb1e_t = cpool.tile([P, HTR], f32)
            nc.sync.dma_start(ffb1e_t[:], ffb1e_d[:])

        # ---- residual x rows (bf16 upload, f32 in SBUF) ----
        xs_t = sbP.tile([P, LT, D], f32, tag="resid", bufs=2, name="xs")
        with tc.tile_pool(name="xsb", bufs=1) as pxb:
            xs_b = pxb.tile([P, LT, D], b16, tag="xs_b")
            nc.sync.dma_start(xs_b[:], xs_d.rearrange("(lt p) d -> p lt d", p=P))
            nc.vector.tensor_copy(xs_t[:], xs_b[:])

        # ---- all weights into SBUF up front (tiny; removes phase-entry stalls)
        def wload(dram, shape, pat, nm):
            t = sbP.tile(shape, b16, tag=nm, bufs=1, name=nm)
            nc.sync.dma_start(t[:], dram.rearrange(pat, dt=DTL) if pat else dram[:])
            return t

        wq1_t = wload(q1W_d, [P, DTL, CS], "p (dt c) -> p dt c", "wq1")
        wk1_t = wload(w1W_d, [P, DTL, CS], "p (dt c) -> p dt c", "wk1")
        ow1_t = wload(o1W_d, [P, D], None, "ow1")
        wq2_t = wload(q2W_d, [P, DTL, CS], "p (dt c) -> p dt c", "wq2")
        wk2_t = wload(w2W_d, [P, DTL, CS], "p (dt c) -> p dt c", "wk2")
        ow2_t = wload(o2W_d, [P, D], None, "ow2")
        w1_t = sbP.tile([P, DTL, MS], b16, tag="w1", bufs=1, name="w1")
        nc.sync.dma_start(w1_t[:], ffW1_d.rearrange("p (dt h) -> p dt h", dt=DTL))
        w2_t = sbP.tile([P, HTR, D], b16, tag="w2", bufs=1, name="w2")
        nc.sync.dma_start(w2_t[:], ffW2_d.rearrange("p (ht d) -> p ht d", ht=HTR))

        def transpose_out(src_nat, ag_in, ag_out):
            """src_nat [P, LT, D] f32 -> bf16 transposed shard, AllGathered
            per d-half so the first AG overlaps the second half's work."""
            with tc.tile_pool(name="tx", bufs=1) as px, \
                 tc.tile_pool(name="tx_ps", bufs=1, space="PSUM") as pp:
                for h in range(2):
                    xt = px.tile([P, DTH, TS], b16, tag="xt", bufs=2)
                    for dtl in range(DTH):
                        dti = h * DTH + dtl
                        tp = pp.tile([P, TS], f32, tag="tp", bufs=2)
                        for lt in range(LT):
                            nc.tensor.transpose(
                                tp[:, bass.ts(lt, P)],
                                src_nat[:, lt, bass.ts(dti, P)], ident_f[:])
                        nc.vector.tensor_copy(xt[:, dtl, :], tp[:])
                    nc.sync.dma_start(
                        ag_in[h].rearrange("(dt p) t -> p dt t", p=P), xt[:])
                    allgather(ag_in[h], ag_out[h])

        def ln_half(rsb_all, st, h, src_dram, resid_nat, pool, extra=None):
            """Load one summed d-half, add residual, take bn stats — emitted
            right after that half's ReduceScatter so it overlaps the other
            half's staging/collective."""
            sa = pool.tile([P, LT, DH], b16, tag=f"sa{h}", bufs=1, name=f"sa{h}")
            nc.sync.dma_start(sa[:], src_dram.rearrange("(lt p) d -> p lt d", p=P))
            half = rsb_all[:, :, h * DH:(h + 1) * DH]
            nc.vector.tensor_copy(half, sa[:])
            nc.vector.tensor_tensor(
                out=half, in0=half,
                in1=resid_nat[:, :, h * DH:(h + 1) * DH], op=ALU.add)
            if extra is not None:
                for lt in range(LT):
                    nc.vector.tensor_tensor(
                        out=rsb_all[:, lt, h * DH:(h + 1) * DH],
                        in0=rsb_all[:, lt, h * DH:(h + 1) * DH],
                        in1=extra[:, h * DH:(h + 1) * DH], op=ALU.add)
            for lt in range(LT):
                nc.vector.bn_stats(st[:, lt, h, :],
                                   rsb_all[:, lt, h * DH:(h + 1) * DH])

        def layernorm_finish(rsb_all, st, out_all, gkey):
            """Aggregate stats (already taken per half) and apply LN."""
            mv = small.tile([P, LT, 2], f32, tag="ln_mv")
            for lt in range(LT):
                nc.vector.bn_aggr(mv[:, lt, :], st[:, lt, :, :])
            t = small.tile([P, LT], f32, tag="ln_t")
            nc.vector.tensor_scalar_add(t[:], mv[:, :, 1], EPS)
            s = small.tile([P, LT], f32, tag="ln_s")
            nc.scalar.sqrt(s[:], t[:])
            r0 = small.tile([P, LT], f32, tag="ln_r0")
            nc.vector.reciprocal(r0[:], s[:])
            # one Newton step: r1 = r0 * (1.5 - 0.5 * t * r0^2)
            u = small.tile([P, LT], f32, tag="ln_u")
            nc.vector.tensor_tensor(out=u[:], in0=t[:], in1=r0[:], op=ALU.mult)
            nc.vector.tensor_tensor(out=u[:], in0=u[:], in1=r0[:], op=ALU.mult)
            nc.vector.tensor_scalar(u[:], u[:], -0.5, 1.5, ALU.mult, ALU.add)
            r1 = small.tile([P, LT], f32, tag="ln_r1")
            nc.vector.tensor_tensor(out=r1[:], in0=r0[:], in1=u[:], op=ALU.mult)
            for lt in range(LT):
                rsb = rsb_all[:, lt, :]
                nc.vector.tensor_scalar(rsb, rsb, mv[:, lt, 0:1], None, ALU.subtract)
                if trivial_affine:
                    nc.vector.tensor_scalar(out_all[:, lt, :], rsb,
                                            r1[:, lt:lt + 1], None, ALU.mult)
                else:
                    g_t = small.tile([P, D], f32, tag="ln_g", bufs=2)
                    nc.sync.dma_start(g_t[:], gb_d["g" + gkey][:])
                    b_t = small.tile([P, D], f32, tag="ln_b", bufs=2)
                    nc.sync.dma_start(b_t[:], gb_d["b" + gkey][:])
                    nc.vector.tensor_scalar(rsb, rsb, r1[:, lt:lt + 1], None,
                                            ALU.mult)
                    nc.vector.tensor_tensor(out=rsb, in0=rsb, in1=g_t[:],
                                            op=ALU.mult)
                    nc.vector.tensor_tensor(out=out_all[:, lt, :], in0=rsb,
                                            in1=b_t[:], op=ALU.add)

        def project(wmat, src_halves, dstf, pool, pp, nm, width=NK):
            """dstf[:, c*width:(c+1)*width] (f32r SBUF, flat) = W^T x^T for
            all 8 AG rank-block chunks of `width` tokens each."""
            for tc_i in range(TC):
                xgs = []
                for h in range(2):
                    xg = pool.tile([P, DTH, width], b16, tag=f"{nm}{h}", bufs=3,
                                   name=f"{nm}{h}")
                    nc.sync.dma_start(
                        xg[:], src_halves[h][tc_i * DH:(tc_i + 1) * DH, :]
                        .rearrange("(dt p) t -> p dt t", p=P))
                    xgs.append(xg)
                ps = pp.tile([P, width], f32, tag=f"ps_{nm}", bufs=3)
                for dti in range(DTL):
                    nc.tensor.matmul(ps[:], wmat[:, dti, :],
                                     xgs[dti // DTH][:, dti % DTH, :],
                                     start=(dti == 0), stop=(dti == DTL - 1))
                nc.vector.tensor_copy(dstf[:, tc_i * width:(tc_i + 1) * width],
                                      ps[:])

        def attention(wq, wk, ow, xTg_halves, kTg_halves, use_mask,
                      resid_nat, x_out, gkey, rs_idx, KT_pre=None,
                      kt_tiles=(MT, MT), tail_fn=None):
            """One TP attention block: Q/K proj for this core's 2 heads over all
            4096 tokens, transposed-score softmax, partial o-proj,
            ReduceScatter (two d-halves), residual + LN on own token shard.

            KT_pre: flat K^T [P, tiles*128] already projected (hoisted).
            kt_tiles: per-batch context m-tile counts (compacted for cross).
            tail_fn(pool, psum_pool): extra work emitted while waiting for this
            block's ReduceScatter (e.g. the next phase's K projection)."""
            offs = (0, kt_tiles[0])
            pA = tc.alloc_tile_pool(name="sbA", bufs=1)
            QT = pA.tile([P, TC, NK], rdt, tag="QT")
            pqk = tc.alloc_tile_pool(name="sbA_qk", bufs=1)
            with tc.tile_pool(name="ps_proj", bufs=1, space="PSUM") as pp:
                if KT_pre is not None:
                    KTf = KT_pre[:]
                    project(wq, xTg_halves, QT[:].rearrange("p a b -> p (a b)"),
                            pqk, pp, "xg")
                elif xTg_halves is kTg_halves:
                    KT = pA.tile([P, TC, NK], rdt, tag="KT")
                    KTf = KT[:].rearrange("p a b -> p (a b)")
                    for tc_i in range(TC):
                        xgs = []
                        for h in range(2):
                            xg = pqk.tile([P, DTH, NK], b16, tag=f"xg{h}",
                                          bufs=3, name=f"xg{h}")
                            nc.sync.dma_start(
                                xg[:], xTg_halves[h][tc_i * DH:(tc_i + 1) * DH, :]
                                .rearrange("(dt p) t -> p dt t", p=P))
                            xgs.append(xg)
                        psq = pp.tile([P, NK], f32, tag="psq", bufs=3)
                        psk = pp.tile([P, NK], f32, tag="psk", bufs=3)
                        for dti in range(DTL):
                            nc.tensor.matmul(psq[:], wq[:, dti, :],
                                             xgs[dti // DTH][:, dti % DTH, :],
                                             start=(dti == 0), stop=(dti == DTL - 1))
                        for dti in range(DTL):
                            nc.tensor.matmul(psk[:], wk[:, dti, :],
                                             xgs[dti // DTH][:, dti % DTH, :],
                                             start=(dti == 0), stop=(dti == DTL - 1))
                        nc.vector.tensor_copy(QT[:, tc_i, :], psq[:])
                        nc.vector.tensor_copy(KT[:, tc_i, :], psk[:])
                else:
                    KT = pA.tile([P, TC, NK], rdt, tag="KT")
                    KTf = KT[:].rearrange("p a b -> p (a b)")
                    project(wq, xTg_halves, QT[:].rearrange("p a b -> p (a b)"),
                            pqk, pp, "xg")
                    project(wk, kTg_halves, KTf, pqk, pp, "kg")
            pqk.release()

            # --- attention core: transposed scores, per batch ---
            ctxT = pA.tile([P, TC, NK], b16, tag="ctxT")
            pat = tc.alloc_tile_pool(name="sbA_at", bufs=1)
            with tc.tile_pool(name="ps_attn", bufs=1, space="PSUM") as pa:
                # prebuild K-natural tiles for both batches so the score/value
                # loops never stall on them
                knats_b = {}
                for b in range(B):
                    tiles = kt_tiles[b]
                    for mh in range((tiles + 3) // 4):
                        rem = min(4, tiles - 4 * mh)
                        kn = pat.tile([P, 4, 2, P], rdt, tag=f"knat{b}_{mh}",
                                      bufs=1, name=f"knat{b}_{mh}")
                        nc.vector.tensor_copy(kn[:, :, :, HD:P], ozcol[:])
                        tp = pa.tile([P, NK], rdt, tag="knt", bufs=2)
                        for j4 in range(rem):
                            gt = offs[b] + 4 * mh + j4   # compacted token tile
                            nc.tensor.transpose(
                                tp[:, bass.ts(j4, P)],
                                KTf[:, bass.ts(gt, P)],
                                ident_r[:])
                        nc.vector.tensor_copy(
                            kn[:, 0:rem, :, 0:HD],
                            tp[:, 0:rem * P].rearrange("p (mt hd) -> p mt hd", hd=P
                                            ).rearrange("p mt (h c) -> p mt h c", c=HD))
                        knats_b[(b, mh)] = kn
                for b in range(B):
                    tiles = kt_tiles[b]
                    for lc in range(4):
                        qc = 4 * b + lc  # query chunk
                        ctxp = [pa.tile([P, NK], f32, tag="ctx", bufs=2,
                                        name=f"ctx{j}") for j in range(2)]
                        for mt in range(tiles):
                            gt = offs[b] + mt
                            s2 = pa.tile([P, 2, NK], f32, tag="s2", bufs=2)
                            for j in range(2):
                                nc.tensor.matmul(
                                    s2[:, j, :],
                                    KTf[bass.ts(j, HD), bass.ts(gt, P)],
                                    QT[bass.ts(j, HD), qc, :],
                                    start=True, stop=True)
                            p2 = pat.tile([P, 2, NK], rdt, tag="p2", bufs=3,
                                          name="p2")
                            bias = maskb_t[:, gt:gt + 1] if use_mask else 0.0
                            nc.scalar.activation(p2[:], s2[:], AF.Exp,
                                                 bias=bias, scale=0.125)
                            for j in range(2):
                                nc.tensor.matmul(
                                    ctxp[j][:],
                                    knats_b[(b, mt // 4)][:, mt % 4, j, :],
                                    p2[:, j, :],
                                    start=(mt == 0), stop=(mt == tiles - 1))
                        for j in range(2):
                            rec = small.tile([1, NK], f32, tag="rec", bufs=2)
                            nc.vector.reciprocal(rec[:], ctxp[j][HD:HD + 1, :])
                            # broadcast 1/Z to rows 64:128 of the same bank
                            nc.tensor.matmul(ctxp[j][HD:2 * HD, :], ones_f[:],
                                             rec[:], start=True, stop=True)
                            recb = small.tile([HD, NK], f32, tag="recb", bufs=2)
                            nc.vector.tensor_copy(recb[:], ctxp[j][HD:2 * HD, :])
                            nc.vector.tensor_tensor(
                                out=ctxT[bass.ts(j, HD), qc, :],
                                in0=ctxp[j][0:HD, :],
                                in1=recb[:], op=ALU.mult)
            pat.release()

            # --- partial output projection -> rs_in halves (dc == d-half),
            # with each half's post-RS residual+stats interleaved ---
            pO = tc.alloc_tile_pool(name="sbA_o", bufs=1)
            rsb_all = pO.tile([P, LT, D], f32, tag="rsb")
            st = small.tile([P, LT, 2, 6], f32, tag="ln_st")
            with tc.tile_pool(name="ps_o", bufs=1, space="PSUM") as po:
                for dc in range(2):
                    for tg in range(T // P // 4):   # groups of 4 token-tiles
                        stg = pO.tile([P, 4, NK], b16, tag="stg", bufs=3)
                        for t4 in range(4):
                            tt = 4 * tg + t4
                            ps = po.tile([P, NK], f32, tag="po", bufs=4)
                            nc.tensor.matmul(
                                ps[:],
                                ctxT[:, tt // 4, bass.ts(tt % 4, P)],
                                ow[:, bass.ts(dc, NK)],
                                start=True, stop=True)
                            # ACT is idle here (exp done) and reads PSUM fast
                            nc.scalar.copy(stg[:, t4, :], ps[:])
                        nc.sync.dma_start(
                            rs_in[rs_idx][dc][tg * 4 * P:(tg + 1) * 4 * P, :]
                            .rearrange("(t4 p) d -> p t4 d", p=P), stg[:])
                    reducescatter(rs_in[rs_idx][dc], rs_out[rs_idx][dc])
                    ln_half(rsb_all, st, dc, rs_out[rs_idx][dc], resid_nat, pO)
                if tail_fn is not None:
                    # fill the ReduceScatter wait with next-phase work
                    tail_fn(pO, po)

            # --- finish LN on own shard ---
            layernorm_finish(rsb_all, st, x_out, gkey)
            pO.release()
            pA.release()

        # ================= self-attention =================
        # x AllGather first — it gates everything; the enc AllGather (needed
        # only at cross-attention) queues behind it on the collective cores
        transpose_out(xs_t, *agx[0])
        with tc.tile_pool(name="encb", bufs=1) as pb:
            for h in range(2):
                eb = pb.tile([P, DTH, SC], b16, tag="eb", bufs=2)
                nc.sync.dma_start(
                    eb[:], encsT_d[h * DH:(h + 1) * DH, :].rearrange(
                        "(dt p) t -> p dt t", p=P))
                nc.sync.dma_start(
                    agenc_in[h].rearrange("(dt p) t -> p dt t", p=P), eb[:])
                allgather(agenc_in[h], agenc_out[h])
        x1 = sbP.tile([P, LT, D], f32, tag="resid", bufs=2, name="x1")
        pK2 = tc.alloc_tile_pool(name="sbK2", bufs=1)
        KT2 = pK2.tile([P, TKC], rdt, tag="KT2", bufs=1, name="KT2")

        def k2_prelude(pool, pp):
            # K2 = enc @ w2W depends only on the early enc AllGather — emit it
            # into self-attention's RS wait window
            project(wk2_t, agenc_out, KT2[:], pool, pp, "k2g", width=SC)

        attention(wq1_t, wk1_t, ow1_t, agx[0][1], agx[0][1], False,
                  xs_t, x1, "1", rs_idx=0, tail_fn=k2_prelude)

        # ================= cross-attention =================
        transpose_out(x1, *agx[1])
        x2 = sbP.tile([P, LT, D], f32, tag="resid", bufs=2, name="x2")
        attention(wq2_t, wk2_t, ow2_t, agx[1][1], agenc_out, True,
                  x1, x2, "2", rs_idx=1, KT_pre=KT2,
                  kt_tiles=(n0p // P, n1p // P))
        pK2.release()

        # ================= FFN =================
        transpose_out(x2, *agx[2])
        pF = tc.alloc_tile_pool(name="sbF", bufs=1)
        w1 = w1_t
        w2 = w2_t
        hT = pF.tile([P, HTR, T], b16, tag="hT")
        with tc.tile_pool(name="ps_ffn", bufs=1, space="PSUM") as pf:
            for tc_i in range(TC):
                xgs = []
                for h in range(2):
                    xg = pF.tile([P, DTH, NK], b16, tag=f"xg2{h}", bufs=3,
                                 name=f"xg2{h}")
                    nc.sync.dma_start(
                        xg[:], agx[2][1][h][tc_i * DH:(tc_i + 1) * DH, :]
                        .rearrange("(dt p) t -> p dt t", p=P))
                    xgs.append(xg)
                for ht in range(HTR):
                    ps = pf.tile([P, NK], f32, tag="ph", bufs=4)
                    for dti in range(DTL):
                        nc.tensor.matmul(ps[:], w1[:, dti, bass.ts(ht, P)],
                                         xgs[dti // DTH][:, dti % DTH, :],
                                         start=(dti == 0), stop=(dti == DTL - 1))
                    # exact gelu: h = (v+b)*0.5 * (1 + erf((v+b)/sqrt(2)))
                    erf_t = pF.tile([P, NK], f32, tag="erf", bufs=2)
                    hv = pF.tile([P, NK], f32, tag="hv", bufs=2)
                    if trivial_ffb:
                        nc.scalar.activation(erf_t[:], ps[:], AF.Erf, scale=SQRT1_2)
                        nc.vector.tensor_scalar(hv[:], ps[:], 0.5, None, ALU.mult)
                    else:
                        nc.scalar.activation(erf_t[:], ps[:], AF.Erf,
                                             bias=ffb1e_t[:, ht:ht + 1], scale=SQRT1_2)
                        nc.vector.tensor_scalar(hv[:], ps[:], ffb1h_t[:, ht:ht + 1],
                                                0.5, ALU.add, ALU.mult)
                    t1 = pF.tile([P, NK], f32, tag="t1", bufs=2)
                    nc.vector.tensor_tensor(out=t1[:], in0=hv[:], in1=erf_t[:],
                                            op=ALU.mult)
                    nc.vector.tensor_tensor(out=hT[:, ht, bass.ts(tc_i, NK)],
                                            in0=t1[:], in1=hv[:], op=ALU.add)
            # ff2 partial -> rs_in[2] halves, post-RS residual+stats interleaved
            x3 = pF.tile([P, LT, D], b16, tag="x3")
            rsb_all = pF.tile([P, LT, D], f32, tag="rsb3")
            st3 = small.tile([P, LT, 2, 6], f32, tag="ln_st")
            if not trivial_ffb:
                ffb2c = pF.tile([P, D], f32, tag="ffb2", bufs=1)
                nc.sync.dma_start(ffb2c[:], ffb2b_d[:])
            for dc in range(2):
                for tg in range(T // P // 4):
                    stg = pF.tile([P, 4, NK], b16, tag="stg2", bufs=3)
                    for t4 in range(4):
                        tt = 4 * tg + t4
                        ps = pf.tile([P, NK], f32, tag="pf2", bufs=4)
                        for ht in range(HTR):
                            nc.tensor.matmul(
                                ps[:],
                                hT[:, ht, bass.ts(tt, P)],
                                w2[:, ht, bass.ts(dc, NK)],
                                start=(ht == 0), stop=(ht == HTR - 1))
                        nc.scalar.copy(stg[:, t4, :], ps[:])
                    nc.sync.dma_start(
                        rs_in[2][dc][tg * 4 * P:(tg + 1) * 4 * P, :]
                        .rearrange("(t4 p) d -> p t4 d", p=P), stg[:])
                reducescatter(rs_in[2][dc], rs_out[2][dc])
                extra = None if trivial_ffb else ffb2c
                ln_half(rsb_all, st3, dc, rs_out[2][dc], x2, pF, extra=extra)
            layernorm_finish(rsb_all, st3, x3, "3")
            for lt in range(LT):   # per-row-tile stores overlap the LN applies
                nc.sync.dma_start(out_d[lt * P:(lt + 1) * P, :], x3[:, lt, :])
        pF.release()

        sbP.release()
        small.release()
        cpool.release()

    lp.__exit__(None, None, None)
    nc.compile()
    return nc


def _pmajor(w, p=P):
    """[R, C] row-major -> [p, (R//p)*C] partition-major tiling."""
    r, c = w.shape
    return np.ascontiguousarray(
        w.reshape(r // p, p, c).swapaxes(0, 1).reshape(p, (r // p) * c))


def _host_prep(inputs):
    x = np.asarray(inputs["x"], np.float32).reshape(T, D)
    enc = np.asarray(inputs["enc_output"], np.float32).reshape(T, D)
    mask = np.asarray(inputs["mask"])

    n = np.arange(D) // HD
    d = np.arange(D) % HD
    perm = d * H + n

    def pw(q, w, o):
        return (np.asarray(q, np.float32)[:, perm],
                np.asarray(w, np.float32)[:, perm],
                np.asarray(o, np.float32)[perm, :])

    q1W, w1W, o1W = pw(inputs["q1W"], inputs["w1W"], inputs["o1W"])
    q2W, w2W, o2W = pw(inputs["q2W"], inputs["w2W"], inputs["o2W"])
    ffW1 = np.asarray(inputs["ffW1"], np.float32)
    ffW2 = np.asarray(inputs["ffW2"], np.float32)
    ffb1 = np.asarray(inputs["ffb1"], np.float32)
    ffb2 = np.asarray(inputs["ffb2"], np.float32)
    g = {k: np.asarray(inputs[k], np.float32)
         for k in ("g1", "b1", "g2", "b2", "g3", "b3")}

    trivial_affine = all(
        np.all(g[f"g{i}"] == 1.0) and np.all(g[f"b{i}"] == 0.0) for i in (1, 2, 3))
    trivial_ffb = bool(np.all(ffb1 == 0.0) and np.all(ffb2 == 0.0))

    # Compact the cross-attention context: masked positions have softmax
    # weight exactly 0 (exp(-1.25e8) == 0) in the reference, so drop them and
    # keep only unmasked enc rows, padded per batch to a multiple of 128.
    # Padded slots get a -1.25e8 exp bias so they also contribute exactly 0.
    kept = [np.where(~mask[b, 0, :, 0])[0] for b in range(B)]
    nps = [max(P, ((len(k) + P - 1) // P) * P) for k in kept]
    n0p, n1p = nps
    tkc = n0p + n1p
    sc = tkc // NCORES
    enc_b = enc.reshape(B, L, D)
    encC = np.zeros((tkc, D), np.float32)
    encC[0:len(kept[0])] = enc_b[0][kept[0]]
    encC[n0p:n0p + len(kept[1])] = enc_b[1][kept[1]]
    biasvec = np.full(tkc, np.float32(-1.25e8), np.float32)
    biasvec[0:len(kept[0])] = 0.0
    biasvec[n0p:n0p + len(kept[1])] = 0.0
    maskb = np.ascontiguousarray(
        biasvec.reshape(tkc // P, P).T).astype(np.float32)

    import ml_dtypes
    b16 = ml_dtypes.bfloat16

    in_maps = []
    for r in range(NCORES):
        tok = slice(r * TS, (r + 1) * TS)
        cs = slice(r * CS, (r + 1) * CS)
        ms = slice(r * MS, (r + 1) * MS)
        im = {
            "xs": np.ascontiguousarray(x[tok]).astype(b16),
            "encsT": np.ascontiguousarray(encC[r * sc:(r + 1) * sc].T).astype(b16),
            "maskb": maskb,
            "q1W": _pmajor(q1W[:, cs]).astype(b16),
            "w1W": _pmajor(w1W[:, cs]).astype(b16),
            "o1W": np.ascontiguousarray(o1W[cs, :]).astype(b16),
            "q2W": _pmajor(q2W[:, cs]).astype(b16),
            "w2W": _pmajor(w2W[:, cs]).astype(b16),
            "o2W": np.ascontiguousarray(o2W[cs, :]).astype(b16),
            "ffW1": _pmajor(ffW1[:, ms]).astype(b16),
            "ffW2": _pmajor(ffW2[ms, :]).astype(b16),
        }
        if not trivial_affine:
            for k in ("g1", "b1", "g2", "b2", "g3", "b3"):
                im[k + "b"] = np.ascontiguousarray(
                    np.broadcast_to(g[k], (P, D)).astype(np.float32))
        if not trivial_ffb:
            im["ffb2b"] = np.ascontiguousarray(
                np.broadcast_to(ffb2, (P, D)).astype(np.float32))
            im["ffb1h"] = np.ascontiguousarray(
                ffb1[ms].reshape(HTR, P).T.astype(np.float32))
            im["ffb1e"] = np.ascontiguousarray(
                (ffb1[ms] * SQRT1_2).reshape(HTR, P).T.astype(np.float32))
        in_maps.append(im)
    return in_maps, trivial_affine, trivial_ffb, n0p, n1p


def kernel(**inputs) -> np.ndarray:
    in_maps, trivial_affine, trivial_ffb, n0p, n1p = _host_prep(inputs)
    key = (trivial_affine, trivial_ffb, n0p, n1p)
    if key not in _PROGRAM_CACHE:
        _PROGRAM_CACHE[key] = _build_program(*key)
    nc = _PROGRAM_CACHE[key]
    res = run_bass_kernel_spmd(nc, in_maps, list(range(NCORES)))
    out = np.empty((T, D), np.float32)
    for r in range(NCORES):
        out[r * TS:(r + 1) * TS, :] = res.results[r]["out"].astype(np.float32)
    return out.reshape(B, L, D)



# revision 17
# speedup vs baseline: 2.5198x; 2.5198x over previous
"""Trainium2 Bass kernel for a transformer decoder layer — 8-way, zero-collective.

Sharding: pure data-parallel over tokens.  Core r owns rows
[512r, 512(r+1)) of the flattened [B*L, D] = [4096, 1024] token axis
(batch 0 = cores 0-3, batch 1 = cores 4-7).  Weights are fully replicated.

Key observation driving the design: in the harness cost model a collective
costs 15us + out_bytes/40GBps, so the TP baseline spent ~1ms of its 1.47ms
in AllGather/ReduceScatter.  Every tensor a core needs besides its own
activations is a kernel *input* (x, enc_output, weights) already present in
HBM, so each core instead recomputes its batch's K projections locally
(~131k extra PE cycles ~ 55us, far cheaper than the collectives) and runs
the whole layer with ZERO collectives:

  - Self-attention: K^T = w1W^T x^T for the core's full batch (redundant
    x4 within a batch group), Q^T for own 512 tokens only, scores/softmax/
    value/o-proj for own queries over all 16 heads, residual+LN — all local.
  - Cross-attention: enc context is compacted host-side (masked positions
    have softmax weight exactly 0: exp(-1.25e8) == 0), padded per batch to
    a common tile count NT; K2^T = w2W^T enc^T computed locally, interleaved
    into the self-attention core where the PE has slack under the ACT-bound
    exp stream.
  - FFN: per-token with full replicated weights, gelu via the ACT table.

Attention value step runs in natural layout: ctx[t, hd] accumulates with
lhsT = p2 (exp scores, [m, t]) and rhs = K-natural tiles [m, 64+1] (ones
column accumulates the softmax denominator Z), so the matmul free dim is
65 instead of a half-wasted 512, and 1/Z applies as a per-partition
tensor_scalar — no PE broadcast dance.

Numerics: bf16 operands on the PE (scores/exp/value/projections), fp32
PSUM accumulation, fp32 residual + LayerNorm.  Host pre-transposes
x^T/enc^T, permutes attention weights head-major, and pre-compacts the
cross-attention context.
"""

import sys

sys.path.insert(0, "/opt/trn_rl_repo")

import numpy as np

import concourse.bass as bass
import concourse.bacc as bacc
import concourse.mybir as mybir
import concourse.tile as tile
from concourse.bass_utils import run_bass_kernel_spmd
from concourse.masks import make_identity

dt = mybir.dt
AF = mybir.ActivationFunctionType
ALU = mybir.AluOpType

P = 128
D = 1024          # d_model
DT = D // P       # 8 input-channel tiles
H = 16            # heads
HD = 64           # head dim
CHT = D // P      # 8 channel tiles (2 heads each)
MLP = 4096
HTT = MLP // P    # 32 hidden tiles
B, L, M = 2, 2048, 2048
T = B * L
NCORES = 8
TS = T // NCORES  # 512 tokens per core
TT = TS // P      # 4 own-token tiles
NK = 512          # matmul free-dim chunk
MT = L // P       # 16 self-attention m-tiles
EPS = 1e-5

_PROGRAM_CACHE = {}


def _build_program(trivial_affine, trivial_ffb, NT):
    """NT: cross-attention context m-tiles (shared across batches; padded
    slots are driven to exactly 0 via the -1.25e8 exp bias)."""
    NC = NT * P           # cross context tokens (padded)
    nc = bacc.Bacc(None)
    f32 = dt.float32
    rdt = dt.float32r
    b16 = dt.bfloat16

    def din(name, shape, d):
        return nc.declare_dram_parameter(name, list(shape), d, isOutput=False)

    xqT_d = din("xqT", [P, DT, NK], b16)    # own x^T (pmajor)
    xbT_d = din("xbT", [P, DT, L], b16)     # full-batch x^T (pmajor)
    xs_d = din("xs", [TS, D], f32)          # own x rows (residual)
    encT_d = din("encT", [P, DT, NC], b16)  # compacted enc^T (pmajor)
    maskb_d = din("maskb", [P, NT], f32)    # 0 or -1.25e8 per context token
    q1W_d = din("q1W", [P, DT, D], b16)
    w1W_d = din("w1W", [P, DT, D], b16)
    o1W_d = din("o1W", [P, DT, D], b16)
    q2W_d = din("q2W", [P, DT, D], b16)
    w2W_d = din("w2W", [P, DT, D], b16)
    o2W_d = din("o2W", [P, DT, D], b16)
    ffW1_d = din("ffW1", [P, DT, MLP], b16)
    ffW2_d = din("ffW2", [P, HTT, D], b16)
    gb_d = {}
    if not trivial_affine:
        for nm in ("g1", "b1", "g2", "b2", "g3", "b3"):
            gb_d[nm] = din(nm + "b", [P, D], f32)
    if not trivial_ffb:
        ffb2b_d = din("ffb2b", [P, D], f32)
        ffb1h_d = din("ffb1h", [P, HTT], f32)
    out_d = nc.declare_dram_parameter("out", [TS, D], f32, isOutput=True)
    import os as _os
    DBG = _os.environ.get("KDBG", "0") == "1"
    if DBG:
        dbg_ctx1_d = nc.declare_dram_parameter("dbg_ctx1", [P, TT, D], b16, isOutput=True)
        dbg_x1_d = nc.declare_dram_parameter("dbg_x1", [P, TT, D], rdt, isOutput=True)
        dbg_x2_d = nc.declare_dram_parameter("dbg_x2", [P, TT, D], rdt, isOutput=True)

    lp = nc.allow_low_precision(reason="bf16 weights/activations")
    lp.__enter__()
    with tile.TileContext(nc) as tc:
        cpool = tc.alloc_tile_pool(name="const", bufs=1)
        small = tc.alloc_tile_pool(name="small", bufs=3)

        ident_f = cpool.tile([P, P], f32)
        make_identity(nc, ident_f[:])
        ident_b = cpool.tile([P, P], b16)
        nc.vector.tensor_copy(ident_b[:], ident_f[:])
        ident_r = cpool.tile([P, P], rdt)
        nc.vector.tensor_copy(ident_r[:], ident_f[:])
        maskb_t = cpool.tile([P, NT], f32)
        nc.sync.dma_start(maskb_t[:], maskb_d[:])
        if not trivial_ffb:
            ffb1h_t = cpool.tile([P, HTT], f32)
            nc.sync.dma_start(ffb1h_t[:], ffb1h_d[:])

        # ---------------- LayerNorm helpers ----------------
        def ln_stats(rsb_tt, st, tt):
            for h in range(2):
                nc.vector.bn_stats(st[:, tt, h, :],
                                   rsb_tt[:, h * NK:(h + 1) * NK])

        def ln_finish(rsb, st, x_out, gkey):
            mv = small.tile([P, TT, 2], f32, tag="ln_mv")
            for tt in range(TT):
                nc.vector.bn_aggr(mv[:, tt, :], st[:, tt, :, :])
            t = small.tile([P, TT], f32, tag="ln_t")
            nc.vector.tensor_scalar_add(t[:], mv[:, :, 1], EPS)
            s = small.tile([P, TT], f32, tag="ln_s")
            nc.scalar.sqrt(s[:], t[:])
            r0 = small.tile([P, TT], f32, tag="ln_r0")
            nc.vector.reciprocal(r0[:], s[:])
            # one Newton step: r1 = r0 * (1.5 - 0.5 * t * r0^2)
            u = small.tile([P, TT], f32, tag="ln_u")
            nc.vector.tensor_tensor(out=u[:], in0=t[:], in1=r0[:], op=ALU.mult)
            nc.vector.tensor_tensor(out=u[:], in0=u[:], in1=r0[:], op=ALU.mult)
            nc.vector.tensor_scalar(u[:], u[:], -0.5, 1.5, ALU.mult, ALU.add)
            r1 = small.tile([P, TT], f32, tag="ln_r1")
            nc.vector.tensor_tensor(out=r1[:], in0=r0[:], in1=u[:], op=ALU.mult)
            for tt in range(TT):
                if trivial_affine:
                    nc.vector.tensor_scalar(
                        x_out[:, tt, :], rsb[:, tt, :], mv[:, tt, 0:1],
                        r1[:, tt:tt + 1], ALU.subtract, ALU.mult)
                else:
                    g_t = small.tile([P, D], f32, tag="ln_g", bufs=2)
                    nc.sync.dma_start(g_t[:], gb_d["g" + gkey][:])
                    b_t = small.tile([P, D], f32, tag="ln_b", bufs=2)
                    nc.sync.dma_start(b_t[:], gb_d["b" + gkey][:])
                    nc.vector.tensor_scalar(
                        rsb[:, tt, :], rsb[:, tt, :], mv[:, tt, 0:1],
                        r1[:, tt:tt + 1], ALU.subtract, ALU.mult)
                    nc.vector.tensor_tensor(out=rsb[:, tt, :], in0=rsb[:, tt, :],
                                            in1=g_t[:], op=ALU.mult)
                    nc.vector.tensor_tensor(out=x_out[:, tt, :], in0=rsb[:, tt, :],
                                            in1=b_t[:], op=ALU.add)

        # ---------------- attention building blocks ----------------
        def proj_T(wt, rhs_t, dst, pp, nm, width, alt=0):
            """dst[:, cht, 0:width] = (W^T x^T) bf16 for all channel tiles."""
            for cht in range(CHT):
                ps = pp.tile([P, NK], f32, tag=f"ps_{nm}", bufs=3)
                for dti in range(DT):
                    nc.tensor.matmul(ps[:, 0:width],
                                     wt[:, dti, cht * P:(cht + 1) * P],
                                     rhs_t[:, dti, 0:width],
                                     start=(dti == 0), stop=(dti == DT - 1))
                if (cht + alt) % 2 == 0:
                    nc.vector.tensor_copy(dst[:, cht, 0:width], ps[:, 0:width])
                else:
                    nc.scalar.copy(dst[:, cht, 0:width], ps[:, 0:width])

        def knat_build(KT_t, kn_t, mt0, ntiles, pp):
            """Transpose KT[:, cht, m-tiles mt0..mt0+ntiles) into K-natural
            tiles kn_t[cht][:, mt, j, 0:64] (col 64 is the preset ones col)."""
            for g0 in range(0, ntiles, 4):
                rem = min(4, ntiles - g0)
                for cht in range(CHT):
                    tp = pp.tile([P, NK], b16, tag="kntp", bufs=2)
                    for j4 in range(rem):
                        mt = mt0 + g0 + j4
                        nc.tensor.transpose(
                            tp[:, j4 * P:(j4 + 1) * P],
                            KT_t[:, cht, mt * P:(mt + 1) * P],
                            ident_b[:])
                    nc.vector.tensor_copy(
                        kn_t[cht][:, mt0 + g0:mt0 + g0 + rem, :, 0:HD],
                        tp[:, 0:rem * P]
                        .rearrange("p (mt hd) -> p mt hd", hd=P)
                        .rearrange("p mt (h c) -> p mt h c", c=HD))

        def attn_core(KT_t, kn_t, QT_t, ctx_nat, n_mt, use_mask, pa, pat,
                      tail_fn=None):
            """Scores + softmax + value for own 512 queries, all 16 heads.
            ctx_nat [P, TT, D] bf16 gets normalized token-natural context.
            tail_fn(hp, pa, pat): filler work emitted per head-pair."""
            for hp in range(CHT):
                ctxp = [pa.tile([P, TT, P], f32, tag=f"ctx{j}", bufs=1,
                                name=f"ctx{j}") for j in range(2)]
                for mt in range(n_mt):
                    s2 = pa.tile([P, 2, NK], f32, tag="s2", bufs=2)
                    for j in range(2):
                        nc.tensor.matmul(
                            s2[:, j, :],
                            KT_t[j * HD:(j + 1) * HD, hp, mt * P:(mt + 1) * P],
                            QT_t[j * HD:(j + 1) * HD, hp, :],
                            start=True, stop=True)
                    p2 = pat.tile([P, 2, NK], b16, tag="p2", bufs=2, name="p2")
                    bias = maskb_t[:, mt:mt + 1] if use_mask else 0.0
                    nc.scalar.activation(p2[:], s2[:], AF.Exp,
                                         bias=bias, scale=0.125)
                    for j in range(2):
                        for ts4 in range(TT):
                            # one start/stop per PSUM bank: start=True marks
                            # the whole 2KB zero region pending-zero, so only
                            # the first group in the bank may set it
                            nc.tensor.matmul(
                                ctxp[j][:, ts4, 0:HD + 1],
                                p2[:, j, ts4 * P:(ts4 + 1) * P],
                                kn_t[hp][:, mt, j, :],
                                start=(mt == 0 and ts4 == 0),
                                stop=(mt == n_mt - 1 and ts4 == TT - 1))
                rec = small.tile([P, 2, TT, 1], f32, tag="rec")
                for j in range(2):
                    nc.vector.reciprocal(rec[:, j], ctxp[j][:, :, HD:HD + 1])
                for j in range(2):
                    hch = (2 * hp + j) * HD
                    for ts4 in range(TT):
                        nc.vector.tensor_scalar(
                            ctx_nat[:, ts4, hch:hch + HD],
                            ctxp[j][:, ts4, 0:HD],
                            rec[:, j, ts4], None, ALU.mult)
                if tail_fn is not None:
                    tail_fn(hp, pa, pat)

        def ctx_transpose(ctx_nat, ctxT, pp):
            for cht in range(CHT):
                tp = pp.tile([P, NK], b16, tag="cttp", bufs=2)
                for ts4 in range(TT):
                    nc.tensor.transpose(
                        tp[:, ts4 * P:(ts4 + 1) * P],
                        ctx_nat[:, ts4, cht * P:(cht + 1) * P],
                        ident_b[:])
                nc.vector.tensor_copy(ctxT[:, cht, :], tp[:])

        def o_proj_ln(ctxT, ow_t, resid, x_out, gkey, pp, rsb, extra=None):
            st = small.tile([P, TT, 2, 6], f32, tag="ln_st")
            for tt in range(TT):
                po = pp.tile([P, D], f32, tag="po", bufs=2)
                for cht in range(CHT):
                    for dc in range(2):
                        nc.tensor.matmul(
                            po[:, dc * NK:(dc + 1) * NK],
                            ctxT[:, cht, tt * P:(tt + 1) * P],
                            ow_t[:, cht, dc * NK:(dc + 1) * NK],
                            start=(cht == 0), stop=(cht == CHT - 1))
                nc.vector.tensor_tensor(out=rsb[:, tt, :], in0=po[:],
                                        in1=resid[:, tt, :], op=ALU.add)
                if extra is not None:
                    nc.vector.tensor_tensor(out=rsb[:, tt, :], in0=rsb[:, tt, :],
                                            in1=extra[:], op=ALU.add)
                ln_stats(rsb[:, tt, :], st, tt)
            ln_finish(rsb, st, x_out, gkey)

        def transpose_own(x_t, xT, pp):
            """x_t [P, TT, D] (f32r) -> xT [P, CHT, TS] bf16."""
            for cht in range(CHT):
                tp = pp.tile([P, NK], rdt, tag="xttp", bufs=2)
                for tt in range(TT):
                    nc.tensor.transpose(
                        tp[:, tt * P:(tt + 1) * P],
                        x_t[:, tt, cht * P:(cht + 1) * P],
                        ident_r[:])
                nc.vector.tensor_copy(xT[:, cht, :], tp[:])

        # ================= phase S: self-attention =================
        # Tile pools form a strict LIFO stack, so reservations are emitted
        # in nesting order (deepest-released first-allocated).
        def wload(pool, dram, shape, nm):
            t = pool.tile(shape, b16, tag=nm, name=nm)
            nc.sync.dma_start(t[:], dram[:])
            return t

        # reserve the p2 tag early so `small` sits at the stack bottom
        small.tile([P, 2, NK], b16, tag="p2", bufs=2, name="p2")

        pPerm = tc.alloc_tile_pool(name="sbPerm", bufs=1)     # dies at end
        x2_t = pPerm.tile([P, TT, D], rdt, tag="x2", name="x2")
        pK2T = tc.alloc_tile_pool(name="sbK2T", bufs=1)       # dies post-o-proj2
        K2T_t = pK2T.tile([P, CHT, NC], b16, tag="K2T", name="K2T")
        ctx1 = pK2T.tile([P, TT, D], b16, tag="ctx_nat", name="ctx1")
        pEnc = tc.alloc_tile_pool(name="sbEnc", bufs=1)       # dies post-self-core
        encT_t = pEnc.tile([P, DT, NC], b16, tag="encT", name="encT")
        nc.sync.dma_start(encT_t[:], encT_d[:])
        w2W_t = wload(pEnc, w2W_d, [P, DT, D], "w2W")

        pS1 = tc.alloc_tile_pool(name="sbS1", bufs=1)         # dies post-self-core
        QT_t = pS1.tile([P, CHT, NK], b16, tag="QT", name="QT")
        K1T_t = pS1.tile([P, CHT, L], b16, tag="K1T", name="K1T")
        kn1 = [pS1.tile([P, MT, 2, HD + 1], b16, tag=f"kn1_{c}", name=f"kn1_{c}")
               for c in range(CHT)]
        for c in range(CHT):
            nc.vector.memset(kn1[c][:, :, :, HD:HD + 1], 1.0)

        pW1 = tc.alloc_tile_pool(name="sbW1", bufs=1)         # dies post-proj
        q1W_t = wload(pW1, q1W_d, [P, DT, D], "q1W")
        w1W_t = wload(pW1, w1W_d, [P, DT, D], "w1W")

        pX = tc.alloc_tile_pool(name="sbX", bufs=1)           # dies post-proj
        XBC = 256  # xbT stream-chunk width (2 m-tiles)
        with tc.tile_pool(name="ps_proj", bufs=1, space="PSUM") as pp:
            xq_t = pX.tile([P, DT, NK], b16, tag="xq", name="xq")
            nc.sync.dma_start(xq_t[:], xqT_d[:])
            proj_T(q1W_t, xq_t, QT_t, pp, "qt", NK)
            for mc in range(L // XBC):
                xb_t = pX.tile([P, DT, XBC], b16, tag="xb", bufs=2, name="xb")
                nc.sync.dma_start(xb_t[:], xbT_d[:, :, mc * XBC:(mc + 1) * XBC])
                for cht in range(CHT):
                    ps = pp.tile([P, NK], f32, tag="ps_k1", bufs=3)
                    for dti in range(DT):
                        nc.tensor.matmul(ps[:, 0:XBC],
                                         w1W_t[:, dti, cht * P:(cht + 1) * P],
                                         xb_t[:, dti, :],
                                         start=(dti == 0), stop=(dti == DT - 1))
                    if cht % 2 == 0:
                        nc.vector.tensor_copy(
                            K1T_t[:, cht, mc * XBC:(mc + 1) * XBC], ps[:, 0:XBC])
                    else:
                        nc.scalar.copy(
                            K1T_t[:, cht, mc * XBC:(mc + 1) * XBC], ps[:, 0:XBC])
                knat_build(K1T_t, kn1, mc * (XBC // P), XBC // P, pp)
        pX.release()
        pW1.release()

        # cross-attention K2^T projection interleaved into the self core as
        # PE filler under the ACT-bound exp stream
        NKC = (NC + NK - 1) // NK

        def k2_tail(hp, pa, pat):
            for ck in range(NKC):
                w = min(NK, NC - ck * NK)
                ps = pa.tile([P, NK], f32, tag="ps_k2", bufs=2, name="ps_k2")
                for dti in range(DT):
                    nc.tensor.matmul(ps[:, 0:w],
                                     w2W_t[:, dti, hp * P:(hp + 1) * P],
                                     encT_t[:, dti, ck * NK:ck * NK + w],
                                     start=(dti == 0), stop=(dti == DT - 1))
                nc.scalar.copy(K2T_t[:, hp, ck * NK:ck * NK + w], ps[:, 0:w])

        with tc.tile_pool(name="ps_attn", bufs=1, space="PSUM") as pa:
            attn_core(K1T_t, kn1, QT_t, ctx1, MT, False, pa, small,
                      tail_fn=k2_tail)
        pS1.release()
        pEnc.release()

        # ---- post-self-core: weights + K-natural tiles for cross ----
        pC2 = tc.alloc_tile_pool(name="sbC2", bufs=1)         # dies post-o-proj2
        o2W_t = wload(pC2, o2W_d, [P, DT, D], "o2W")
        q2W_t = wload(pC2, q2W_d, [P, DT, D], "q2W")
        kn2 = [pC2.tile([P, NT, 2, HD + 1], b16, tag=f"kn2_{c}", name=f"kn2_{c}")
               for c in range(CHT)]
        for c in range(CHT):
            nc.vector.memset(kn2[c][:, :, :, HD:HD + 1], 1.0)
        with tc.tile_pool(name="ps_kn2", bufs=1, space="PSUM") as pk:
            knat_build(K2T_t, kn2, 0, NT, pk)

        # ---- o-proj + LN1 + x1^T + Q2^T ----
        pO1ph = tc.alloc_tile_pool(name="sbO1ph", bufs=1)     # dies post-Q2T
        xs_t = pO1ph.tile([P, TT, D], f32, tag="xs", name="xs")
        nc.sync.dma_start(xs_t[:], xs_d.rearrange("(lt p) d -> p lt d", p=P))
        o1W_t = wload(pO1ph, o1W_d, [P, DT, D], "o1W")
        ctxT1 = pO1ph.tile([P, CHT, TS], b16, tag="ctxT1", name="ctxT1")
        rsb1 = pO1ph.tile([P, TT, D], f32, tag="rsb1", name="rsb1")
        x1_t = pC2.tile([P, TT, D], rdt, tag="x1", name="x1")
        with tc.tile_pool(name="ps_o1", bufs=1, space="PSUM") as po:
            ctx_transpose(ctx1, ctxT1, po)
            o_proj_ln(ctxT1, o1W_t, xs_t, x1_t, "1", po, rsb1)
            x1T_t = pO1ph.tile([P, CHT, TS], b16, tag="x1T", name="x1T")
            transpose_own(x1_t, x1T_t, po)
        if DBG:
            nc.sync.dma_start(dbg_ctx1_d[:], ctx1[:])
            nc.sync.dma_start(dbg_x1_d[:], x1_t[:])
        Q2T_t = pC2.tile([P, CHT, NK], b16, tag="Q2T", name="Q2T")
        with tc.tile_pool(name="ps_proj2", bufs=1, space="PSUM") as pp:
            proj_T(q2W_t, x1T_t, Q2T_t, pp, "q2t", NK, alt=1)
        pO1ph.release()

        # ================= phase C: cross-attention =================
        ctx2 = pC2.tile([P, TT, D], b16, tag="ctx_nat2", name="ctx2")
        with tc.tile_pool(name="ps_attn2", bufs=1, space="PSUM") as pa:
            attn_core(K2T_t, kn2, Q2T_t, ctx2, NT, True, pa, small)

        pO2ph = tc.alloc_tile_pool(name="sbO2ph", bufs=1)     # dies post-o-proj2
        ctxT2 = pO2ph.tile([P, CHT, TS], b16, tag="ctxT2", name="ctxT2")
        rsb2 = pO2ph.tile([P, TT, D], f32, tag="rsb2", name="rsb2")
        with tc.tile_pool(name="ps_o2", bufs=1, space="PSUM") as po:
            ctx_transpose(ctx2, ctxT2, po)
            o_proj_ln(ctxT2, o2W_t, x1_t, x2_t, "2", po, rsb2)
        if DBG:
            nc.sync.dma_start(dbg_x2_d[:], x2_t[:])
        pO2ph.release()
        pC2.release()
        pK2T.release()

        # ================= phase F: FFN =================
        pFm = tc.alloc_tile_pool(name="sbFm", bufs=1)         # dies at end
        x2T_t = pFm.tile([P, CHT, TS], b16, tag="x2T", name="x2T")
        with tc.tile_pool(name="ps_x2t", bufs=1, space="PSUM") as po:
            transpose_own(x2_t, x2T_t, po)
        hT_t = pFm.tile([P, HTT, TS], b16, tag="hT", name="hT")
        w2_t = pFm.tile([P, HTT, D], b16, tag="ffW2", name="ffW2")
        rsb3 = pFm.tile([P, TT, D], f32, tag="rsb3", name="rsb3")
        if not trivial_ffb:
            ffb2c = pFm.tile([P, D], f32, tag="ffb2", name="ffb2")
            nc.sync.dma_start(ffb2c[:], ffb2b_d[:])
        pFw1 = tc.alloc_tile_pool(name="sbFw1", bufs=1)       # dies post-ff1
        w1_t = pFw1.tile([P, DT, MLP], b16, tag="ffW1", name="ffW1")
        for wc in range(4):  # chunked so ff1 starts after the first quarter
            nc.sync.dma_start(w1_t[:, :, wc * D:(wc + 1) * D],
                              ffW1_d[:, :, wc * D:(wc + 1) * D])
        with tc.tile_pool(name="ps_ffn", bufs=1, space="PSUM") as pf:
            for ht in range(HTT):
                if ht == HTT // 2:
                    # W1 half consumed; start the W2 fetch mid-ff1
                    nc.sync.dma_start(w2_t[:], ffW2_d[:])
                ps = pf.tile([P, NK], f32, tag="ph", bufs=3)
                for dti in range(DT):
                    nc.tensor.matmul(ps[:],
                                     w1_t[:, dti, ht * P:(ht + 1) * P],
                                     x2T_t[:, dti, :],
                                     start=(dti == 0), stop=(dti == DT - 1))
                bias = 0.0 if trivial_ffb else ffb1h_t[:, ht:ht + 1]
                nc.scalar.activation(hT_t[:, ht, :], ps[:], AF.Gelu, bias=bias)
        pFw1.release()
        st3 = small.tile([P, TT, 2, 6], f32, tag="ln_st")
        with tc.tile_pool(name="ps_ff2", bufs=1, space="PSUM") as pf:
            for tt in range(TT):
                pff = pf.tile([P, D], f32, tag="pf2", bufs=2)
                for ht in range(HTT):
                    for dc in range(2):
                        nc.tensor.matmul(
                            pff[:, dc * NK:(dc + 1) * NK],
                            hT_t[:, ht, tt * P:(tt + 1) * P],
                            w2_t[:, ht, dc * NK:(dc + 1) * NK],
                            start=(ht == 0), stop=(ht == HTT - 1))
                nc.vector.tensor_tensor(out=rsb3[:, tt, :], in0=pff[:],
                                        in1=x2_t[:, tt, :], op=ALU.add)
                if not trivial_ffb:
                    nc.vector.tensor_tensor(out=rsb3[:, tt, :],
                                            in0=rsb3[:, tt, :],
                                            in1=ffb2c[:], op=ALU.add)
                ln_stats(rsb3[:, tt, :], st3, tt)
            ln_finish(rsb3, st3, rsb3, "3")  # LN applied in place
            for tt in range(TT):
                nc.sync.dma_start(out_d[tt * P:(tt + 1) * P, :], rsb3[:, tt, :])
        pFm.release()
        pK2T_dummy = None  # (stack: pFm popped; below: perm pools)
        pPerm.release()
        small.release()
        cpool.release()

    lp.__exit__(None, None, None)
    nc.compile()
    return nc


def _pmajor(w, p=P):
    """[R, C] row-major -> [p, R//p, C] partition-major tiling."""
    r, c = w.shape
    return np.ascontiguousarray(w.reshape(r // p, p, c).swapaxes(0, 1))


def _host_prep(inputs):
    import ml_dtypes

    b16 = ml_dtypes.bfloat16
    x = np.asarray(inputs["x"], np.float32)          # [B, L, D]
    enc = np.asarray(inputs["enc_output"], np.float32)
    mask = np.asarray(inputs["mask"])                # [B, 1, M, 1]

    n = np.arange(D) // HD
    d = np.arange(D) % HD
    perm = d * H + n

    def pw(q, w, o):
        return (np.asarray(q, np.float32)[:, perm],
                np.asarray(w, np.float32)[:, perm],
                np.asarray(o, np.float32)[perm, :])

    q1W, w1W, o1W = pw(inputs["q1W"], inputs["w1W"], inputs["o1W"])
    q2W, w2W, o2W = pw(inputs["q2W"], inputs["w2W"], inputs["o2W"])
    ffW1 = np.asarray(inputs["ffW1"], np.float32)
    ffW2 = np.asarray(inputs["ffW2"], np.float32)
    ffb1 = np.asarray(inputs["ffb1"], np.float32)
    ffb2 = np.asarray(inputs["ffb2"], np.float32)
    g = {k: np.asarray(inputs[k], np.float32)
         for k in ("g1", "b1", "g2", "b2", "g3", "b3")}

    trivial_affine = all(
        np.all(g[f"g{i}"] == 1.0) and np.all(g[f"b{i}"] == 0.0) for i in (1, 2, 3))
    trivial_ffb = bool(np.all(ffb1 == 0.0) and np.all(ffb2 == 0.0))

    # Compact the cross-attention context per batch: masked positions have
    # softmax weight exactly 0 in the reference (exp(-1.25e8) == 0).  Both
    # batches are padded to a common m-tile count NT so the program is
    # identical across cores; padded slots get the -1.25e8 exp bias.
    kept = [np.where(~mask[b, 0, :, 0])[0] for b in range(B)]
    NT = max(1, max((len(k) + P - 1) // P for k in kept))
    ncx = NT * P
    encCT = []
    maskbs = []
    for b in range(B):
        encC = np.zeros((ncx, D), np.float32)
        encC[0:len(kept[b])] = enc[b][kept[b]]
        biasvec = np.full(ncx, np.float32(-1.25e8), np.float32)
        biasvec[0:len(kept[b])] = 0.0
        encCT.append(np.ascontiguousarray(
            _pmajor(np.ascontiguousarray(encC.T))).astype(b16))
        maskbs.append(np.ascontiguousarray(
            biasvec.reshape(NT, P).T).astype(np.float32))

    xbT = [np.ascontiguousarray(
        _pmajor(np.ascontiguousarray(x[b].T))).astype(b16) for b in range(B)]

    wmaps = {
        "q1W": _pmajor(q1W).astype(b16), "w1W": _pmajor(w1W).astype(b16),
        "o1W": _pmajor(o1W).astype(b16),
        "q2W": _pmajor(q2W).astype(b16), "w2W": _pmajor(w2W).astype(b16),
        "o2W": _pmajor(o2W).astype(b16),
        "ffW1": _pmajor(ffW1).astype(b16), "ffW2": _pmajor(ffW2).astype(b16),
    }
    if not trivial_affine:
        for k in ("g1", "b1", "g2", "b2", "g3", "b3"):
            wmaps[k + "b"] = np.ascontiguousarray(
                np.broadcast_to(g[k], (P, D)).astype(np.float32))

    in_maps = []
    for r in range(NCORES):
        b = r // (NCORES // B)
        lo = (r % (NCORES // B)) * TS
        xq = np.ascontiguousarray(x[b, lo:lo + TS].T)  # [D, TS]
        im = dict(wmaps)
        im["xqT"] = np.ascontiguousarray(_pmajor(xq)).astype(b16)
        im["xbT"] = xbT[b]
        im["xs"] = np.ascontiguousarray(x[b, lo:lo + TS]).astype(np.float32)
        im["encT"] = encCT[b]
        im["maskb"] = maskbs[b]
        if not trivial_ffb:
            im["ffb2b"] = np.ascontiguousarray(
                np.broadcast_to(ffb2, (P, D)).astype(np.float32))
            im["ffb1h"] = np.ascontiguousarray(
                ffb1.reshape(HTT, P).T.astype(np.float32))
        in_maps.append(im)
    return in_maps, trivial_affine, trivial_ffb, NT


def kernel(**inputs) -> np.ndarray:
    in_maps, trivial_affine, trivial_ffb, NT = _host_prep(inputs)
    key = (trivial_affine, trivial_ffb, NT)
    if key not in _PROGRAM_CACHE:
        _PROGRAM_CACHE[key] = _build_program(*key)
    nc = _PROGRAM_CACHE[key]
    res = run_bass_kernel_spmd(nc, in_maps, list(range(NCORES)))
    out = np.empty((T, D), np.float32)
    for r in range(NCORES):
        out[r * TS:(r + 1) * TS, :] = res.results[r]["out"].astype(np.float32)
    return out.reshape(B, L, D)


# revision 22
# speedup vs baseline: 2.5210x; 1.0005x over previous
"""Trainium2 Bass kernel for a transformer decoder layer — 8-way, zero-collective.

Sharding: pure data-parallel over tokens.  Core r owns rows
[512r, 512(r+1)) of the flattened [B*L, D] = [4096, 1024] token axis
(batch 0 = cores 0-3, batch 1 = cores 4-7).  Weights are fully replicated.

Key observation driving the design: in the harness cost model a collective
costs 15us + out_bytes/40GBps, so the TP baseline spent ~1ms of its 1.47ms
in AllGather/ReduceScatter.  Every tensor a core needs besides its own
activations is a kernel *input* (x, enc_output, weights) already present in
HBM, so each core instead recomputes its batch's K projections locally
(~131k extra PE cycles ~ 55us, far cheaper than the collectives) and runs
the whole layer with ZERO collectives:

  - Self-attention: K^T = w1W^T x^T for the core's full batch (redundant
    x4 within a batch group), Q^T for own 512 tokens only, scores/softmax/
    value/o-proj for own queries over all 16 heads, residual+LN — all local.
  - Cross-attention: enc context is compacted host-side (masked positions
    have softmax weight exactly 0: exp(-1.25e8) == 0), padded per batch to
    a common tile count NT; K2^T = w2W^T enc^T computed locally, interleaved
    into the self-attention core where the PE has slack under the ACT-bound
    exp stream.
  - FFN: per-token with full replicated weights, gelu via the ACT table.

Attention value step runs in natural layout: ctx[t, hd] accumulates with
lhsT = p2 (exp scores, [m, t]) and rhs = K-natural tiles [m, 64+1] (ones
column accumulates the softmax denominator Z), so the matmul free dim is
65 instead of a half-wasted 512, and 1/Z applies as a per-partition
tensor_scalar — no PE broadcast dance.

Numerics: bf16 operands on the PE (scores/exp/value/projections), fp32
PSUM accumulation, fp32 residual + LayerNorm.  Host pre-transposes
x^T/enc^T, permutes attention weights head-major, and pre-compacts the
cross-attention context.
"""

import sys

sys.path.insert(0, "/opt/trn_rl_repo")

import numpy as np

import concourse.bass as bass
import concourse.bacc as bacc
import concourse.mybir as mybir
import concourse.tile as tile
from concourse.bass_utils import run_bass_kernel_spmd
from concourse.masks import make_identity

dt = mybir.dt
AF = mybir.ActivationFunctionType
ALU = mybir.AluOpType

P = 128
D = 1024          # d_model
DT = D // P       # 8 input-channel tiles
H = 16            # heads
HD = 64           # head dim
CHT = D // P      # 8 channel tiles (2 heads each)
MLP = 4096
HTT = MLP // P    # 32 hidden tiles
B, L, M = 2, 2048, 2048
T = B * L
NCORES = 8
TS = T // NCORES  # 512 tokens per core
TT = TS // P      # 4 own-token tiles
NK = 512          # matmul free-dim chunk
MT = L // P       # 16 self-attention m-tiles
EPS = 1e-5

_PROGRAM_CACHE = {}


def _build_program(trivial_affine, trivial_ffb, NT):
    """NT: cross-attention context m-tiles (shared across batches; padded
    slots are driven to exactly 0 via the -1.25e8 exp bias)."""
    NC = NT * P           # cross context tokens (padded)
    nc = bacc.Bacc(None)
    f32 = dt.float32
    rdt = dt.float32r
    b16 = dt.bfloat16

    def din(name, shape, d):
        return nc.declare_dram_parameter(name, list(shape), d, isOutput=False)

    xqT_d = din("xqT", [P, DT, NK], b16)    # own x^T (pmajor)
    xbT_d = din("xbT", [P, DT, L], b16)     # full-batch x^T (pmajor)
    xs_d = din("xs", [TS, D], f32)          # own x rows (residual)
    encT_d = din("encT", [P, DT, NC], b16)  # compacted enc^T (pmajor)
    maskb_d = din("maskb", [P, NT], f32)    # 0 or -1.25e8 per context token
    q1W_d = din("q1W", [P, DT, D], b16)
    w1W_d = din("w1W", [P, DT, D], b16)
    o1W_d = din("o1W", [P, DT, D], b16)
    q2W_d = din("q2W", [P, DT, D], b16)
    w2W_d = din("w2W", [P, DT, D], b16)
    o2W_d = din("o2W", [P, DT, D], b16)
    ffW1_d = din("ffW1", [P, DT, MLP], b16)
    ffW2_d = din("ffW2", [P, HTT, D], b16)
    gb_d = {}
    if not trivial_affine:
        for nm in ("g1", "b1", "g2", "b2", "g3", "b3"):
            gb_d[nm] = din(nm + "b", [P, D], f32)
    if not trivial_ffb:
        ffb2b_d = din("ffb2b", [P, D], f32)
        ffb1h_d = din("ffb1h", [P, HTT], f32)
    out_d = nc.declare_dram_parameter("out", [TS, D], f32, isOutput=True)
    import os as _os
    DBG = _os.environ.get("KDBG", "0") == "1"
    if DBG:
        dbg_ctx1_d = nc.declare_dram_parameter("dbg_ctx1", [P, TT, D], b16, isOutput=True)
        dbg_x1_d = nc.declare_dram_parameter("dbg_x1", [P, TT, D], rdt, isOutput=True)
        dbg_x2_d = nc.declare_dram_parameter("dbg_x2", [P, TT, D], rdt, isOutput=True)

    lp = nc.allow_low_precision(reason="bf16 weights/activations")
    lp.__enter__()
    with tile.TileContext(nc) as tc:
        cpool = tc.alloc_tile_pool(name="const", bufs=1)
        small = tc.alloc_tile_pool(name="small", bufs=3)

        ident_f = cpool.tile([P, P], f32)
        make_identity(nc, ident_f[:])
        ident_b = cpool.tile([P, P], b16)
        nc.vector.tensor_copy(ident_b[:], ident_f[:])
        ident_r = cpool.tile([P, P], rdt)
        nc.vector.tensor_copy(ident_r[:], ident_f[:])
        maskb_t = cpool.tile([P, NT], f32)
        nc.sync.dma_start(maskb_t[:], maskb_d[:])
        if not trivial_ffb:
            ffb1h_t = cpool.tile([P, HTT], f32)
            nc.sync.dma_start(ffb1h_t[:], ffb1h_d[:])

        # ---------------- LayerNorm helpers ----------------
        def ln_stats(rsb_tt, st, tt):
            for h in range(2):
                nc.vector.bn_stats(st[:, tt, h, :],
                                   rsb_tt[:, h * NK:(h + 1) * NK])

        def ln_finish(rsb, st, x_out, gkey):
            mv = small.tile([P, TT, 2], f32, tag="ln_mv")
            for tt in range(TT):
                nc.vector.bn_aggr(mv[:, tt, :], st[:, tt, :, :])
            t = small.tile([P, TT], f32, tag="ln_t")
            nc.vector.tensor_scalar_add(t[:], mv[:, :, 1], EPS)
            s = small.tile([P, TT], f32, tag="ln_s")
            nc.scalar.sqrt(s[:], t[:])
            r0 = small.tile([P, TT], f32, tag="ln_r0")
            nc.vector.reciprocal(r0[:], s[:])
            # one Newton step: r1 = r0 * (1.5 - 0.5 * t * r0^2)
            u = small.tile([P, TT], f32, tag="ln_u")
            nc.vector.tensor_tensor(out=u[:], in0=t[:], in1=r0[:], op=ALU.mult)
            nc.vector.tensor_tensor(out=u[:], in0=u[:], in1=r0[:], op=ALU.mult)
            nc.vector.tensor_scalar(u[:], u[:], -0.5, 1.5, ALU.mult, ALU.add)
            r1 = small.tile([P, TT], f32, tag="ln_r1")
            nc.vector.tensor_tensor(out=r1[:], in0=r0[:], in1=u[:], op=ALU.mult)
            for tt in range(TT):
                if trivial_affine:
                    nc.vector.tensor_scalar(
                        x_out[:, tt, :], rsb[:, tt, :], mv[:, tt, 0:1],
                        r1[:, tt:tt + 1], ALU.subtract, ALU.mult)
                else:
                    g_t = small.tile([P, D], f32, tag="ln_g", bufs=2)
                    nc.sync.dma_start(g_t[:], gb_d["g" + gkey][:])
                    b_t = small.tile([P, D], f32, tag="ln_b", bufs=2)
                    nc.sync.dma_start(b_t[:], gb_d["b" + gkey][:])
                    nc.vector.tensor_scalar(
                        rsb[:, tt, :], rsb[:, tt, :], mv[:, tt, 0:1],
                        r1[:, tt:tt + 1], ALU.subtract, ALU.mult)
                    nc.vector.tensor_tensor(out=rsb[:, tt, :], in0=rsb[:, tt, :],
                                            in1=g_t[:], op=ALU.mult)
                    nc.vector.tensor_tensor(out=x_out[:, tt, :], in0=rsb[:, tt, :],
                                            in1=b_t[:], op=ALU.add)

        # ---------------- attention building blocks ----------------
        def proj_T(wt, rhs_t, dst, pp, nm, width, alt=0):
            """dst[:, cht, 0:width] = (W^T x^T) bf16 for all channel tiles."""
            for cht in range(CHT):
                ps = pp.tile([P, NK], f32, tag=f"ps_{nm}", bufs=3)
                for dti in range(DT):
                    nc.tensor.matmul(ps[:, 0:width],
                                     wt[:, dti, cht * P:(cht + 1) * P],
                                     rhs_t[:, dti, 0:width],
                                     start=(dti == 0), stop=(dti == DT - 1))
                if (cht + alt) % 2 == 0:
                    nc.vector.tensor_copy(dst[:, cht, 0:width], ps[:, 0:width])
                else:
                    nc.scalar.copy(dst[:, cht, 0:width], ps[:, 0:width])

        def knat_build(KT_t, kn_t, mt0, ntiles, pp):
            """Transpose KT[:, cht, m-tiles mt0..mt0+ntiles) into K-natural
            tiles kn_t[cht][:, mt, j, 0:64] (col 64 is the preset ones col)."""
            for g0 in range(0, ntiles, 4):
                rem = min(4, ntiles - g0)
                for cht in range(CHT):
                    tp = pp.tile([P, NK], b16, tag="kntp", bufs=2)
                    for j4 in range(rem):
                        mt = mt0 + g0 + j4
                        nc.tensor.transpose(
                            tp[:, j4 * P:(j4 + 1) * P],
                            KT_t[:, cht, mt * P:(mt + 1) * P],
                            ident_b[:])
                    nc.vector.tensor_copy(
                        kn_t[cht][:, mt0 + g0:mt0 + g0 + rem, :, 0:HD],
                        tp[:, 0:rem * P]
                        .rearrange("p (mt hd) -> p mt hd", hd=P)
                        .rearrange("p mt (h c) -> p mt h c", c=HD))

        def attn_core(KT_t, kn_t, QT_t, ctx_nat, n_mt, use_mask, pa, pat,
                      filler=None, gap_ns=350.0):
            """Scores + softmax + value for own 512 queries, all 16 heads.
            ctx_nat [P, TT, D] bf16 gets normalized token-natural context.
            filler: generator that emits one PE work unit per next() and
            yields its ns cost — pulled into the per-mt ACT-bound idle gap."""
            credit = 0.0
            for hp in range(CHT):
                ctxp = [pa.tile([P, TT, P], f32, tag=f"ctx{j}", bufs=1,
                                name=f"ctx{j}") for j in range(2)]
                for mt in range(n_mt):
                    s2 = pa.tile([P, 2, NK], f32, tag="s2", bufs=2)
                    for j in range(2):
                        nc.tensor.matmul(
                            s2[:, j, :],
                            KT_t[j * HD:(j + 1) * HD, hp, mt * P:(mt + 1) * P],
                            QT_t[j * HD:(j + 1) * HD, hp, :],
                            start=True, stop=True)
                    p2 = pat.tile([P, 2, NK], b16, tag="p2", bufs=2, name="p2")
                    bias = maskb_t[:, mt:mt + 1] if use_mask else 0.0
                    nc.scalar.activation(p2[:], s2[:], AF.Exp,
                                         bias=bias, scale=0.125)
                    for j in range(2):
                        for ts4 in range(TT):
                            # one start/stop per PSUM bank: start=True marks
                            # the whole 2KB zero region pending-zero, so only
                            # the first group in the bank may set it
                            nc.tensor.matmul(
                                ctxp[j][:, ts4, 0:HD + 1],
                                p2[:, j, ts4 * P:(ts4 + 1) * P],
                                kn_t[hp][:, mt, j, :],
                                start=(mt == 0 and ts4 == 0),
                                stop=(mt == n_mt - 1 and ts4 == TT - 1))
                    if filler is not None:
                        credit += gap_ns
                        while credit > 0:
                            c = next(filler, None)
                            if c is None:
                                filler = None
                                break
                            credit -= c
                rec = small.tile([P, 2, TT, 1], f32, tag="rec")
                for j in range(2):
                    nc.vector.reciprocal(rec[:, j], ctxp[j][:, :, HD:HD + 1])
                for j in range(2):
                    hch = (2 * hp + j) * HD
                    for ts4 in range(TT):
                        nc.vector.tensor_scalar(
                            ctx_nat[:, ts4, hch:hch + HD],
                            ctxp[j][:, ts4, 0:HD],
                            rec[:, j, ts4], None, ALU.mult)
            if filler is not None:
                while next(filler, None) is not None:
                    pass

        def ctx_transpose(ctx_nat, ctxT, pp):
            for cht in range(CHT):
                tp = pp.tile([P, NK], b16, tag="cttp", bufs=2)
                for ts4 in range(TT):
                    nc.tensor.transpose(
                        tp[:, ts4 * P:(ts4 + 1) * P],
                        ctx_nat[:, ts4, cht * P:(cht + 1) * P],
                        ident_b[:])
                nc.vector.tensor_copy(ctxT[:, cht, :], tp[:])

        def o_proj_ln(ctxT, ow_t, resid, x_out, gkey, pp, rsb, extra=None):
            st = small.tile([P, TT, 2, 6], f32, tag="ln_st")
            for tt in range(TT):
                po = pp.tile([P, D], f32, tag="po", bufs=2)
                for cht in range(CHT):
                    for dc in range(2):
                        nc.tensor.matmul(
                            po[:, dc * NK:(dc + 1) * NK],
                            ctxT[:, cht, tt * P:(tt + 1) * P],
                            ow_t[:, cht, dc * NK:(dc + 1) * NK],
                            start=(cht == 0), stop=(cht == CHT - 1))
                nc.vector.tensor_tensor(out=rsb[:, tt, :], in0=po[:],
                                        in1=resid[:, tt, :], op=ALU.add)
                if extra is not None:
                    nc.vector.tensor_tensor(out=rsb[:, tt, :], in0=rsb[:, tt, :],
                                            in1=extra[:], op=ALU.add)
                ln_stats(rsb[:, tt, :], st, tt)
            ln_finish(rsb, st, x_out, gkey)

        def transpose_own(x_t, xT, pp):
            """x_t [P, TT, D] (f32r) -> xT [P, CHT, TS] bf16."""
            for cht in range(CHT):
                tp = pp.tile([P, NK], rdt, tag="xttp", bufs=2)
                for tt in range(TT):
                    nc.tensor.transpose(
                        tp[:, tt * P:(tt + 1) * P],
                        x_t[:, tt, cht * P:(cht + 1) * P],
                        ident_r[:])
                nc.vector.tensor_copy(xT[:, cht, :], tp[:])

        # ================= phase S: self-attention =================
        # Tile pools form a strict LIFO stack, so reservations are emitted
        # in nesting order (deepest-released first-allocated).
        def wload(pool, dram, shape, nm):
            t = pool.tile(shape, b16, tag=nm, name=nm)
            nc.sync.dma_start(t[:], dram[:])
            return t

        # reserve the p2 tag early so `small` sits at the stack bottom
        small.tile([P, 2, NK], b16, tag="p2", bufs=2, name="p2")

        pPerm = tc.alloc_tile_pool(name="sbPerm", bufs=1)     # dies at end
        x2_t = pPerm.tile([P, TT, D], rdt, tag="x2", name="x2")
        pK2T = tc.alloc_tile_pool(name="sbK2T", bufs=1)       # dies post-o-proj2
        K2T_t = pK2T.tile([P, CHT, NC], b16, tag="K2T", name="K2T")
        ctx1 = pK2T.tile([P, TT, D], b16, tag="ctx_nat", name="ctx1")
        pPre = tc.alloc_tile_pool(name="sbPre", bufs=1)       # dies post-o-proj2
        xs_t = pPre.tile([P, TT, D], f32, tag="xs", name="xs")
        pEnc = tc.alloc_tile_pool(name="sbEnc", bufs=1)       # dies post-self-core
        encT_t = pEnc.tile([P, DT, NC], b16, tag="encT", name="encT")
        w2W_t = pEnc.tile([P, DT, D], b16, tag="w2W", name="w2W")

        pS1 = tc.alloc_tile_pool(name="sbS1", bufs=1)         # dies post-self-core
        QT_t = pS1.tile([P, CHT, NK], b16, tag="QT", name="QT")
        K1T_t = pS1.tile([P, CHT, L], b16, tag="K1T", name="K1T")
        kn1 = [pS1.tile([P, MT, 2, HD + 1], b16, tag=f"kn1_{c}", name=f"kn1_{c}")
               for c in range(CHT)]
        for c in range(CHT):
            nc.vector.memset(kn1[c][:, :, :, HD:HD + 1], 1.0)

        # DMA order is start-latency-critical: xq + q1W first so the Q
        # projection starts ASAP, then w1W + the first xb chunks; encT/w2W/
        # xs/maskb ride behind (consumed mid-core or later).
        pQ1 = tc.alloc_tile_pool(name="sbQ1", bufs=1)         # dies post-QT-proj
        xq_t = pQ1.tile([P, DT, NK], b16, tag="xq", name="xq")
        nc.sync.dma_start(xq_t[:], xqT_d[:])
        q1W_t = wload(pQ1, q1W_d, [P, DT, D], "q1W")

        XBC = 256  # xbT stream-chunk width (2 m-tiles)
        with tc.tile_pool(name="ps_proj", bufs=1, space="PSUM") as pp:
            proj_T(q1W_t, xq_t, QT_t, pp, "qt", NK)
        pQ1.release()
        pW1 = tc.alloc_tile_pool(name="sbW1", bufs=1)         # dies post-proj
        w1W_t = wload(pW1, w1W_d, [P, DT, D], "w1W")
        pX = tc.alloc_tile_pool(name="sbX", bufs=1)           # dies post-proj
        with tc.tile_pool(name="ps_proj1b", bufs=1, space="PSUM") as pp:
            for mc in range(L // XBC):
                xb_t = pX.tile([P, DT, XBC], b16, tag="xb", bufs=2, name="xb")
                nc.sync.dma_start(xb_t[:], xbT_d[:, :, mc * XBC:(mc + 1) * XBC])
                if mc == 2:
                    # slot the mid-core/late loads behind the hot chunks
                    nc.sync.dma_start(encT_t[:], encT_d[:])
                    nc.sync.dma_start(w2W_t[:], w2W_d[:])
                if mc == 4:
                    nc.sync.dma_start(
                        xs_t[:], xs_d.rearrange("(lt p) d -> p lt d", p=P))
                for cht in range(CHT):
                    ps = pp.tile([P, NK], f32, tag="ps_k1", bufs=3)
                    for dti in range(DT):
                        nc.tensor.matmul(ps[:, 0:XBC],
                                         w1W_t[:, dti, cht * P:(cht + 1) * P],
                                         xb_t[:, dti, :],
                                         start=(dti == 0), stop=(dti == DT - 1))
                    if cht % 2 == 0:
                        nc.vector.tensor_copy(
                            K1T_t[:, cht, mc * XBC:(mc + 1) * XBC], ps[:, 0:XBC])
                    else:
                        nc.scalar.copy(
                            K1T_t[:, cht, mc * XBC:(mc + 1) * XBC], ps[:, 0:XBC])
                knat_build(K1T_t, kn1, mc * (XBC // P), XBC // P, pp)
        pX.release()
        pW1.release()

        # cross-attention K2^T projection, streamed one matmul at a time into
        # the self core's per-mt PE idle gaps (the core is ACT/exp-bound)
        NKC = (NC + NK - 1) // NK

        def k2_filler(pa):
            for cht in range(CHT):
                for ck in range(NKC):
                    w = min(NK, NC - ck * NK)
                    ps = pa.tile([P, NK], f32, tag="ps_k2", bufs=2,
                                 name="ps_k2")
                    for dti in range(DT):
                        nc.tensor.matmul(
                            ps[:, 0:w],
                            w2W_t[:, dti, cht * P:(cht + 1) * P],
                            encT_t[:, dti, ck * NK:ck * NK + w],
                            start=(dti == 0), stop=(dti == DT - 1))
                        yield w * 0.417
                    nc.vector.tensor_copy(
                        K2T_t[:, cht, ck * NK:ck * NK + w], ps[:, 0:w])
                    yield 0.0

        with tc.tile_pool(name="ps_attn", bufs=1, space="PSUM") as pa:
            attn_core(K1T_t, kn1, QT_t, ctx1, MT, False, pa, small,
                      filler=k2_filler(pa))
        pS1.release()
        pEnc.release()

        # ---- post-self-core: weights + K-natural tiles for cross ----
        pC2 = tc.alloc_tile_pool(name="sbC2", bufs=1)         # dies post-o-proj2
        kn2 = [pC2.tile([P, NT, 2, HD + 1], b16, tag=f"kn2_{c}", name=f"kn2_{c}")
               for c in range(CHT)]
        x1_t = pC2.tile([P, TT, D], rdt, tag="x1", name="x1")
        pO1ph = tc.alloc_tile_pool(name="sbO1ph", bufs=1)     # dies post-Q2T
        o1W_t = wload(pO1ph, o1W_d, [P, DT, D], "o1W")
        q2W_t = wload(pC2, q2W_d, [P, DT, D], "q2W")
        o2W_t = wload(pC2, o2W_d, [P, DT, D], "o2W")
        for c in range(CHT):
            nc.vector.memset(kn2[c][:, :, :, HD:HD + 1], 1.0)

        # ---- knat2 + o-proj + LN1 + x1^T + Q2^T ----
        ctxT1 = pO1ph.tile([P, CHT, TS], b16, tag="ctxT1", name="ctxT1")
        rsb1 = pO1ph.tile([P, TT, D], f32, tag="rsb1", name="rsb1")
        with tc.tile_pool(name="ps_kn2", bufs=1, space="PSUM") as pk:
            knat_build(K2T_t, kn2, 0, NT, pk)
            ctx_transpose(ctx1, ctxT1, pk)
        with tc.tile_pool(name="ps_o1", bufs=1, space="PSUM") as po:
            o_proj_ln(ctxT1, o1W_t, xs_t, x1_t, "1", po, rsb1)
            x1T_t = pO1ph.tile([P, CHT, TS], b16, tag="x1T", name="x1T")
            transpose_own(x1_t, x1T_t, po)
        if DBG:
            nc.sync.dma_start(dbg_ctx1_d[:], ctx1[:])
            nc.sync.dma_start(dbg_x1_d[:], x1_t[:])
        Q2T_t = pC2.tile([P, CHT, NK], b16, tag="Q2T", name="Q2T")
        with tc.tile_pool(name="ps_proj2", bufs=1, space="PSUM") as pp:
            proj_T(q2W_t, x1T_t, Q2T_t, pp, "q2t", NK, alt=1)
        pO1ph.release()

        # ================= phase C: cross-attention =================
        ctx2 = pC2.tile([P, TT, D], b16, tag="ctx_nat2", name="ctx2")
        with tc.tile_pool(name="ps_attn2", bufs=1, space="PSUM") as pa:
            attn_core(K2T_t, kn2, Q2T_t, ctx2, NT, True, pa, small)

        pO2ph = tc.alloc_tile_pool(name="sbO2ph", bufs=1)     # dies post-o-proj2
        ctxT2 = pO2ph.tile([P, CHT, TS], b16, tag="ctxT2", name="ctxT2")
        rsb2 = pO2ph.tile([P, TT, D], f32, tag="rsb2", name="rsb2")
        with tc.tile_pool(name="ps_o2", bufs=1, space="PSUM") as po:
            ctx_transpose(ctx2, ctxT2, po)
            o_proj_ln(ctxT2, o2W_t, x1_t, x2_t, "2", po, rsb2)
        if DBG:
            nc.sync.dma_start(dbg_x2_d[:], x2_t[:])
        pO2ph.release()
        pC2.release()
        pPre.release()
        pK2T.release()

        # ================= phase F: FFN =================
        pFm = tc.alloc_tile_pool(name="sbFm", bufs=1)         # dies at end
        x2T_t = pFm.tile([P, CHT, TS], b16, tag="x2T", name="x2T")
        with tc.tile_pool(name="ps_x2t", bufs=1, space="PSUM") as po:
            transpose_own(x2_t, x2T_t, po)
        hT_t = pFm.tile([P, HTT, TS], b16, tag="hT", name="hT")
        w2_t = pFm.tile([P, HTT, D], b16, tag="ffW2", name="ffW2")
        rsb3 = pFm.tile([P, TT, D], f32, tag="rsb3", name="rsb3")
        if not trivial_ffb:
            ffb2c = pFm.tile([P, D], f32, tag="ffb2", name="ffb2")
            nc.sync.dma_start(ffb2c[:], ffb2b_d[:])
        pFw1 = tc.alloc_tile_pool(name="sbFw1", bufs=1)       # dies post-ff1
        w1_t = pFw1.tile([P, DT, MLP], b16, tag="ffW1", name="ffW1")
        for wc in range(4):  # chunked so ff1 starts after the first quarter
            nc.sync.dma_start(w1_t[:, :, wc * D:(wc + 1) * D],
                              ffW1_d[:, :, wc * D:(wc + 1) * D])
        with tc.tile_pool(name="ps_ffn", bufs=1, space="PSUM") as pf:
            for ht in range(HTT):
                if ht == HTT // 2:
                    # W1 half consumed; start the W2 fetch mid-ff1
                    nc.sync.dma_start(w2_t[:], ffW2_d[:])
                ps = pf.tile([P, NK], f32, tag="ph", bufs=3)
                for dti in range(DT):
                    nc.tensor.matmul(ps[:],
                                     w1_t[:, dti, ht * P:(ht + 1) * P],
                                     x2T_t[:, dti, :],
                                     start=(dti == 0), stop=(dti == DT - 1))
                bias = 0.0 if trivial_ffb else ffb1h_t[:, ht:ht + 1]
                nc.scalar.activation(hT_t[:, ht, :], ps[:], AF.Gelu, bias=bias)
        pFw1.release()
        st3 = small.tile([P, TT, 2, 6], f32, tag="ln_st")
        with tc.tile_pool(name="ps_ff2", bufs=1, space="PSUM") as pf:
            for tt in range(TT):
                pff = pf.tile([P, D], f32, tag="pf2", bufs=2)
                for ht in range(HTT):
                    for dc in range(2):
                        nc.tensor.matmul(
                            pff[:, dc * NK:(dc + 1) * NK],
                            hT_t[:, ht, tt * P:(tt + 1) * P],
                            w2_t[:, ht, dc * NK:(dc + 1) * NK],
                            start=(ht == 0), stop=(ht == HTT - 1))
                nc.vector.tensor_tensor(out=rsb3[:, tt, :], in0=pff[:],
                                        in1=x2_t[:, tt, :], op=ALU.add)
                if not trivial_ffb:
                    nc.vector.tensor_tensor(out=rsb3[:, tt, :],
                                            in0=rsb3[:, tt, :],
                                            in1=ffb2c[:], op=ALU.add)
                ln_stats(rsb3[:, tt, :], st3, tt)
            ln_finish(rsb3, st3, rsb3, "3")  # LN applied in place
            for tt in range(TT):
                nc.sync.dma_start(out_d[tt * P:(tt + 1) * P, :], rsb3[:, tt, :])
        pFm.release()
        pK2T_dummy = None  # (stack: pFm popped; below: perm pools)
        pPerm.release()
        small.release()
        cpool.release()

    lp.__exit__(None, None, None)
    nc.compile()
    return nc


def _pmajor(w, p=P):
    """[R, C] row-major -> [p, R//p, C] partition-major tiling."""
    r, c = w.shape
    return np.ascontiguousarray(w.reshape(r // p, p, c).swapaxes(0, 1))


def _host_prep(inputs):
    import ml_dtypes

    b16 = ml_dtypes.bfloat16
    x = np.asarray(inputs["x"], np.float32)          # [B, L, D]
    enc = np.asarray(inputs["enc_output"], np.float32)
    mask = np.asarray(inputs["mask"])                # [B, 1, M, 1]

    n = np.arange(D) // HD
    d = np.arange(D) % HD
    perm = d * H + n

    def pw(q, w, o):
        return (np.asarray(q, np.float32)[:, perm],
                np.asarray(w, np.float32)[:, perm],
                np.asarray(o, np.float32)[perm, :])

    q1W, w1W, o1W = pw(inputs["q1W"], inputs["w1W"], inputs["o1W"])
    q2W, w2W, o2W = pw(inputs["q2W"], inputs["w2W"], inputs["o2W"])
    ffW1 = np.asarray(inputs["ffW1"], np.float32)
    ffW2 = np.asarray(inputs["ffW2"], np.float32)
    ffb1 = np.asarray(inputs["ffb1"], np.float32)
    ffb2 = np.asarray(inputs["ffb2"], np.float32)
    g = {k: np.asarray(inputs[k], np.float32)
         for k in ("g1", "b1", "g2", "b2", "g3", "b3")}

    trivial_affine = all(
        np.all(g[f"g{i}"] == 1.0) and np.all(g[f"b{i}"] == 0.0) for i in (1, 2, 3))
    trivial_ffb = bool(np.all(ffb1 == 0.0) and np.all(ffb2 == 0.0))

    # Compact the cross-attention context per batch: masked positions have
    # softmax weight exactly 0 in the reference (exp(-1.25e8) == 0).  Both
    # batches are padded to a common m-tile count NT so the program is
    # identical across cores; padded slots get the -1.25e8 exp bias.
    kept = [np.where(~mask[b, 0, :, 0])[0] for b in range(B)]
    NT = max(1, max((len(k) + P - 1) // P for k in kept))
    ncx = NT * P
    encCT = []
    maskbs = []
    for b in range(B):
        encC = np.zeros((ncx, D), np.float32)
        encC[0:len(kept[b])] = enc[b][kept[b]]
        biasvec = np.full(ncx, np.float32(-1.25e8), np.float32)
        biasvec[0:len(kept[b])] = 0.0
        encCT.append(np.ascontiguousarray(
            _pmajor(np.ascontiguousarray(encC.T))).astype(b16))
        maskbs.append(np.ascontiguousarray(
            biasvec.reshape(NT, P).T).astype(np.float32))

    xbT = [np.ascontiguousarray(
        _pmajor(np.ascontiguousarray(x[b].T))).astype(b16) for b in range(B)]

    wmaps = {
        "q1W": _pmajor(q1W).astype(b16), "w1W": _pmajor(w1W).astype(b16),
        "o1W": _pmajor(o1W).astype(b16),
        "q2W": _pmajor(q2W).astype(b16), "w2W": _pmajor(w2W).astype(b16),
        "o2W": _pmajor(o2W).astype(b16),
        "ffW1": _pmajor(ffW1).astype(b16), "ffW2": _pmajor(ffW2).astype(b16),
    }
    if not trivial_affine:
        for k in ("g1", "b1", "g2", "b2", "g3", "b3"):
            wmaps[k + "b"] = np.ascontiguousarray(
                np.broadcast_to(g[k], (P, D)).astype(np.float32))

    in_maps = []
    for r in range(NCORES):
        b = r // (NCORES // B)
        lo = (r % (NCORES // B)) * TS
        xq = np.ascontiguousarray(x[b, lo:lo + TS].T)  # [D, TS]
        im = dict(wmaps)
        im["xqT"] = np.ascontiguousarray(_pmajor(xq)).astype(b16)
        im["xbT"] = xbT[b]
        im["xs"] = np.ascontiguousarray(x[b, lo:lo + TS]).astype(np.float32)
        im["encT"] = encCT[b]
        im["maskb"] = maskbs[b]
        if not trivial_ffb:
            im["ffb2b"] = np.ascontiguousarray(
                np.broadcast_to(ffb2, (P, D)).astype(np.float32))
            im["ffb1h"] = np.ascontiguousarray(
                ffb1.reshape(HTT, P).T.astype(np.float32))
        in_maps.append(im)
    return in_maps, trivial_affine, trivial_ffb, NT


def kernel(**inputs) -> np.ndarray:
    in_maps, trivial_affine, trivial_ffb, NT = _host_prep(inputs)
    key = (trivial_affine, trivial_ffb, NT)
    if key not in _PROGRAM_CACHE:
        _PROGRAM_CACHE[key] = _build_program(*key)
    nc = _PROGRAM_CACHE[key]
    res = run_bass_kernel_spmd(nc, in_maps, list(range(NCORES)))
    out = np.empty((T, D), np.float32)
    for r in range(NCORES):
        out[r * TS:(r + 1) * TS, :] = res.results[r]["out"].astype(np.float32)
    return out.reshape(B, L, D)
